# revision 1
# baseline (speedup 1.0000x reference)
"""FPQuantLinear (MXFP4 pseudo-quant linear) Trainium2 kernel.

y = einsum('bsk,nk->bsn', Q(x), Q(w)) + bias
Q = per-32-group Hadamard rotation + MXFP4 (e2m1 + power-of-2 block scale)
quant-dequant with abs_max scaling and a global scale.

8 NeuronCores, token-parallel; W quantization n-sharded + AllGather:
  - Host: H = s*P (P = +-1 exact in fp16); x,w pre-scaled by gs*s, split into
    2 fp16 halves, pre-transposed K-major. Rotation = 2 fp16 matmuls against
    the +-1 block-diagonal (exact to ~2^-22).
  - scale = 2^ceil(log2(absmax/6+1e-30)) via integer exponent tricks; absmax
    over each k-group's 32 partitions via DVE 32x32 transpose + free-reduce.
  - e2m1 RTN: r = (v + cs) - cs, cs = sign(v)|C, C = 1.5*2^22*max(ef(v),sc).
    r = dq*gs exactly; exact in fp8e4m3 for this data.
  - Main matmul fp8 (DoubleRow): yT[n,t] += wdqT.T @ xdqT over k, PSUM fp32;
    evac with *1/(gsx*gsw) + bias on ACT; host reassembles.
"""
import os

import numpy as np

GROUP = 32
B, S, K, N = 4, 2048, 4096, 4096
TOK = B * S
NCORES = 8
TPC = TOK // NCORES      # tokens per core
NPC = N // NCORES        # n-columns quantized per core

_prog_cache = {}
LAST_EXEC_NS = None
LAST_RESULTS = None


def _build_program(n_splits):
    import concourse.bass as bass
    import concourse.mybir as mybir
    import concourse.tile as tile
    from concourse import bacc

    F32 = mybir.dt.float32
    F16 = mybir.dt.float16
    FP8 = mybir.dt.float8e4
    I32 = mybir.dt.int32
    Alu = mybir.AluOpType
    Act = mybir.ActivationFunctionType

    KB = K // 128          # 32 k-blocks
    NT = N // 128          # 32 n-tiles
    FDC = 1024             # max quant chunk free-dim
    use_dr = os.environ.get("KQ_DR", "1") == "1"
    use_gp_tt = os.environ.get("KQ_GPTT", "0") == "1"
    w_shard = os.environ.get("KQ_WSHARD", "1") == "1"
    dbg = os.environ.get("KQ_DEBUG", "0") == "1"

    nc = bacc.Bacc(None, target_bir_lowering=False)

    xh_d = nc.dram_tensor("xhT", [K, TPC], F16, kind="ExternalInput")
    xl_d = nc.dram_tensor("xlT", [K, TPC], F16, kind="ExternalInput")
    wn = NPC if w_shard else N
    wh_d = nc.dram_tensor("whT", [K, wn], F16, kind="ExternalInput")
    wl_d = nc.dram_tensor("wlT", [K, wn], F16, kind="ExternalInput")
    bdp_d = nc.dram_tensor("bdp", [n_splits, 128, 128], F16, kind="ExternalInput")
    bias_d = nc.dram_tensor("bias", [N], F32, kind="ExternalInput")
    scl_d = nc.dram_tensor("scl", [128, 1], F32, kind="ExternalInput")
    y_d = nc.dram_tensor("yT", [N, TPC], F32, kind="ExternalOutput")

    if w_shard:
        wsh_d = nc.dram_tensor("wdq_sh", [KB, 128, NPC], FP8)
        wg_d = nc.dram_tensor("wdq_g", [NCORES, KB, 128, NPC], FP8,
                              addr_space="Shared")
        wdq_view = wg_d
    else:
        wsh_d = None
        wdq_view = nc.dram_tensor("wdq_i", [KB, 128, N], FP8)
    xdq_dbg = nc.dram_tensor("xdq_i", [128, KB, TPC], FP8,
                             kind="ExternalOutput") if dbg else None
    _ = use_gp_tt

    with tile.TileContext(nc) as tc:
        with (
            tc.tile_pool(name="singles", bufs=1) as singles,
            tc.tile_pool(name="stage", bufs=3) as stage,
            tc.tile_pool(name="qwork", bufs=2) as qwork,
            tc.tile_pool(name="dqout", bufs=3) as dqout,
            tc.tile_pool(name="wmain", bufs=3) as wmain,
            tc.tile_pool(name="ymain", bufs=3) as ymain,
            tc.tile_pool(name="vps", bufs=2, space="PSUM") as vps,
            tc.tile_pool(name="yps", bufs=2, space="PSUM") as yps,
        ):
            # ---------- constants ----------
            bdp_t = singles.tile([128, n_splits, 128], F16)
            for sp in range(n_splits):
                nc.sync.dma_start(bdp_t[:, sp, :], bdp_d[sp])
            bias_t = singles.tile([128, NT], F32)
            nc.sync.dma_start(bias_t[:], bias_d[:].rearrange("(a b) -> b a", b=128))
            scl_bc = singles.tile([128, 1], F32)
            nc.sync.dma_start(scl_bc[:], scl_d[:])
            sgnmask = singles.tile([128, 1], I32)
            nc.vector.memset(sgnmask[:], -0x80000000)

            inv6 = float(np.float32(1.0) / np.float32(6.0))

            # ---------- quant pipeline for one [128, fd] chunk ----------
            def quant_chunk(h_t, l_t, fd, dq_out_ap):
                vp = vps.tile([128, FDC], F32, tag="vp")
                for j in range(fd // 512):
                    sl = slice(j * 512, (j + 1) * 512)
                    for sp in range(n_splits):
                        nc.tensor.matmul(
                            vp[:, sl], bdp_t[:, sp, :], (h_t, l_t)[sp % 2][:, sl],
                            start=(sp == 0), stop=(sp == n_splits - 1),
                        )
                vc = qwork.tile([128, FDC], F32, tag="vc")
                nc.scalar.activation(vc[:, :fd], vp[:, :fd], Act.Copy)
                # absmax over 32-partition groups via transpose32 + reduce
                t32 = qwork.tile([128, FDC], F32, tag="t32")
                nc.vector.transpose(t32[:, :fd], vc[:, :fd])
                red = qwork.tile([128, FDC // 32], F32, tag="red")
                nc.vector.tensor_reduce(
                    red[:, : fd // 32],
                    t32[:, :fd].rearrange("p (j b) -> p j b", b=32),
                    mybir.AxisListType.X, Alu.max,
                    apply_absolute_value=True,
                )
                # scale on the reduced tensor (small): sc = ceilpow2(red/6+eps)
                rs = fd // 32
                t2 = qwork.tile([128, FDC // 32], F32, tag="t2")
                nc.vector.tensor_scalar(
                    t2[:, :rs], red[:, :rs], inv6, 1e-30, Alu.mult, Alu.add
                )
                nc.vector.tensor_scalar(
                    t2[:, :rs].bitcast(I32), t2[:, :rs].bitcast(I32),
                    0x7FFFFF, None, Alu.add,
                )
                nc.vector.tensor_scalar(
                    t2[:, :rs].bitcast(I32), t2[:, :rs].bitcast(I32),
                    0x7F800000, None, Alu.bitwise_and,
                )
                scb = t2[:, :rs].unsqueeze(2).broadcast_to([128, rs, 32])
                t32r = t32[:, :fd].rearrange("p (j b) -> p j b", b=32)
                # rest of the chain in the transposed32 domain
                ea = qwork.tile([128, FDC], F32, tag="ea")
                nc.vector.tensor_scalar(
                    ea[:, :fd].bitcast(I32), t32[:, :fd].bitcast(I32),
                    0x7F800000, None, Alu.bitwise_and,
                )
                e = qwork.tile([128, FDC], F32, tag="e")
                nc.vector.tensor_tensor(
                    e[:, :fd].rearrange("p (j b) -> p j b", b=32).bitcast(I32),
                    ea[:, :fd].rearrange("p (j b) -> p j b", b=32).bitcast(I32),
                    scb.bitcast(I32), Alu.max,
                )
                C = qwork.tile([128, FDC], F32, tag="C")
                nc.vector.tensor_scalar(
                    C[:, :fd].bitcast(I32), e[:, :fd].bitcast(I32),
                    0x0B400000, None, Alu.add,
                )
                cs = qwork.tile([128, FDC], F32, tag="cs")
                nc.vector.scalar_tensor_tensor(
                    cs[:, :fd].bitcast(I32), t32[:, :fd].bitcast(I32),
                    sgnmask[:, 0:1], C[:, :fd].bitcast(I32),
                    Alu.bitwise_and, Alu.bitwise_or,
                )
                t = qwork.tile([128, FDC], F32, tag="t")
                nc.vector.tensor_tensor(t[:, :fd], t32[:, :fd], cs[:, :fd], Alu.add)
                dq8t = qwork.tile([128, FDC], FP8, tag="dq8t")
                nc.vector.tensor_tensor(dq8t[:, :fd], t[:, :fd], cs[:, :fd],
                                        Alu.subtract)
                nc.vector.transpose(dq_out_ap, dq8t[:, :fd])

            # ---------- phase 1: quantize this core's W n-slice ----------
            wfd = min(FDC, wn)
            for kb in range(KB):
                for ch in range(wn // wfd):
                    nsl = slice(ch * wfd, (ch + 1) * wfd)
                    h_t = stage.tile([128, wfd], F16, tag="wh")
                    l_t = stage.tile([128, wfd], F16, tag="wl")
                    nc.sync.dma_start(h_t[:], wh_d[kb * 128:(kb + 1) * 128, nsl])
                    nc.sync.dma_start(l_t[:], wl_d[kb * 128:(kb + 1) * 128, nsl])
                    dq_t = dqout.tile([128, wfd], FP8, tag="wdq")
                    quant_chunk(h_t, l_t, wfd, dq_t[:])
                    if w_shard:
                        nc.sync.dma_start(wsh_d[kb, :, nsl], dq_t[:])
                    else:
                        nc.sync.dma_start(wdq_view[kb, :, nsl], dq_t[:])

            # ---------- all-gather W shards ----------
            if w_shard:
                nc.gpsimd.collective_compute(
                    "AllGather", mybir.AluOpType.bypass,
                    replica_groups=[list(range(NCORES))],
                    ins=[wsh_d[:]],
                    outs=[wg_d[:]],
                )

            # ---------- phases 2+3 interleaved: X-quant token-chunks, then
            # main matmul for that chunk (PE) overlapping next chunk quant (DVE)
            NSPLIT = int(os.environ.get("KQ_TSPLIT", "2"))
            TW = TPC // NSPLIT
            xdq_ts = [singles.tile([128, KB, TW], FP8, name=f"xdq{i}", tag=f"xdq{i}")
                      for i in range(NSPLIT)]
            for tci in range(NSPLIT):
                xdq_c = xdq_ts[tci]
                toff = tci * TW
                for kb in range(KB):
                    h_t = stage.tile([128, TW], F16, tag="xh")
                    l_t = stage.tile([128, TW], F16, tag="xl")
                    nc.sync.dma_start(
                        h_t[:], xh_d[kb * 128:(kb + 1) * 128, toff:toff + TW])
                    nc.sync.dma_start(
                        l_t[:], xl_d[kb * 128:(kb + 1) * 128, toff:toff + TW])
                    quant_chunk(h_t, l_t, TW, xdq_c[:, kb, :])
                if dbg:
                    nc.sync.dma_start(xdq_dbg[:, :, toff:toff + TW], xdq_c[:])
                for nt in range(NT):
                    wnt = wmain.tile([128, KB, 128], FP8, tag="wnt")
                    if w_shard:
                        c = nt // (NPC // 128)
                        off = (nt % (NPC // 128)) * 128
                        src = wg_d[c, :, :, off:off + 128]
                    else:
                        src = wdq_view[:, :, nt * 128:(nt + 1) * 128]
                    nc.sync.dma_start(wnt[:], src.rearrange("kb p n -> p kb n"))
                    for tch in range(TW // 512):
                        tsl = slice(tch * 512, (tch + 1) * 512)
                        gsl = slice(toff + tch * 512, toff + (tch + 1) * 512)
                        yp = yps.tile([128, 512], F32, tag="yp")
                        if use_dr:
                            for kb2 in range(KB // 2):
                                nc.tensor.matmul(
                                    yp[:],
                                    wnt[:, 2 * kb2:2 * kb2 + 2, :],
                                    xdq_c[:, 2 * kb2:2 * kb2 + 2, tsl],
                                    start=(kb2 == 0), stop=(kb2 == KB // 2 - 1),
                                    perf_mode=mybir.MatmulPerfMode.DoubleRow,
                                )
                        else:
                            for kb in range(KB):
                                nc.tensor.matmul(
                                    yp[:], wnt[:, kb, :], xdq_c[:, kb, tsl],
                                    start=(kb == 0), stop=(kb == KB - 1),
                                )
                        ysb = ymain.tile([128, 512], F32, tag="ysb")
                        nc.scalar.activation(
                            ysb[:], yp[:], Act.Identity,
                            bias=bias_t[:, nt:nt + 1], scale=scl_bc[:, 0:1],
                        )
                        nc.sync.dma_start(y_d[nt * 128:(nt + 1) * 128, gsl], ysb[:])

    nc.compile()
    return nc


def _get_program(n_splits):
    key = (n_splits, os.environ.get("KQ_DR", "1"), os.environ.get("KQ_GPTT", "0"),
           os.environ.get("KQ_WSHARD", "1"), os.environ.get("KQ_DEBUG", "0"),
           os.environ.get("KQ_TSPLIT", "2"))
    if key not in _prog_cache:
        _prog_cache[key] = _build_program(n_splits)
    return _prog_cache[key]


def _prepare(x, weight, bias, hadamard_matrix, weight_global_scale, act_global_scale):
    x = np.asarray(x, dtype=np.float32)
    weight = np.asarray(weight, dtype=np.float32)
    bias = np.asarray(bias, dtype=np.float32)
    H = np.asarray(hadamard_matrix, dtype=np.float32)
    gsw = np.float32(np.asarray(weight_global_scale).reshape(-1)[0])
    gsx = np.float32(np.asarray(act_global_scale).reshape(-1)[0])

    s = np.float32(np.abs(H).max())
    Pm = (H / s).astype(np.float32)
    Ph = Pm.astype(np.float16)
    Pl = (Pm - Ph.astype(np.float32)).astype(np.float16)
    generic = bool(np.any(Pl != 0))
    n_splits = 4 if generic else 2

    eye4 = np.eye(4, dtype=np.float32)
    if generic:
        bdp = np.stack([
            np.kron(eye4, Ph.astype(np.float32)).astype(np.float16),
            np.kron(eye4, Ph.astype(np.float32)).astype(np.float16),
            np.kron(eye4, Pl.astype(np.float32)).astype(np.float16),
            np.kron(eye4, Pl.astype(np.float32)).astype(np.float16),
        ])
    else:
        bdp = np.stack([np.kron(eye4, Ph.astype(np.float32)).astype(np.float16)] * 2)

    cx = np.float32(gsx * s)
    cw = np.float32(gsw * s)
    inv_gs = np.full((128, 1), np.float32(1.0) / np.float32(gsx * gsw),
                     dtype=np.float32)

    xs = (x.reshape(TOK, K) * cx).astype(np.float32)
    xh = xs.astype(np.float16)
    xl = (xs - xh.astype(np.float32)).astype(np.float16)

    ws = (weight * cw).astype(np.float32)
    wh = ws.astype(np.float16)
    wl = (ws - wh.astype(np.float32)).astype(np.float16)
    whT = np.ascontiguousarray(wh.T)
    wlT = np.ascontiguousarray(wl.T)

    nc = _get_program(n_splits)

    w_shard = os.environ.get("KQ_WSHARD", "1") == "1"
    in_maps = []
    for c in range(NCORES):
        tsl = slice(c * TPC, (c + 1) * TPC)
        nslc = slice(c * NPC, (c + 1) * NPC)
        in_maps.append({
            "xhT": np.ascontiguousarray(xh[tsl].T),
            "xlT": np.ascontiguousarray(xl[tsl].T),
            "whT": np.ascontiguousarray(whT[:, nslc]) if w_shard else whT,
            "wlT": np.ascontiguousarray(wlT[:, nslc]) if w_shard else wlT,
            "bdp": bdp,
            "bias": bias,
            "scl": inv_gs,
        })
    return nc, in_maps


def _assemble(results):
    y = np.empty((TOK, N), dtype=np.float32)
    for c in range(NCORES):
        y[c * TPC:(c + 1) * TPC] = results[c]["yT"].T
    return y.reshape(B, S, N)


def kernel(x, weight, bias, hadamard_matrix, weight_global_scale, act_global_scale):
    from concourse.bass_utils import run_bass_kernel_spmd

    nc, in_maps = _prepare(x, weight, bias, hadamard_matrix,
                           weight_global_scale, act_global_scale)
    trace = os.environ.get("KQ_TRACE", "0") == "1"
    res = run_bass_kernel_spmd(nc, in_maps, list(range(NCORES)), trace=trace,
                               tmpdir=os.environ.get("KQ_TRACE_DIR") or None)
    if trace:
        global LAST_EXEC_NS
        LAST_EXEC_NS = res.exec_time_ns
        print("exec_time_ns:", res.exec_time_ns, "mean:", res.mean_exec_time_ns)
    global LAST_RESULTS
    LAST_RESULTS = res.results
    return _assemble(res.results)



# revision 10
# speedup vs baseline: 1.1813x; 1.1813x over previous
"""FPQuantLinear (MXFP4 pseudo-quant linear) Trainium2 kernel.

y = einsum('bsk,nk->bsn', Q(x), Q(w)) + bias
Q = per-32-group Hadamard rotation + MXFP4 (e2m1 + power-of-2 block scale)
quant-dequant with abs_max scaling and a global scale.

8 NeuronCores, token-parallel; W quantization n-sharded + AllGather:
  - Host: H = s*P (P = +-1 exact in fp16); x,w pre-scaled by gs*s, split into
    2 fp16 halves, pre-transposed K-major. Rotation = 2 fp16 matmuls against
    the +-1 block-diagonal (exact to ~2^-22).
  - Quant chain in fp16 (KQ_CHAIN=16, default) for 2x DVE throughput, or f32
    (KQ_CHAIN=32, bit-exact vs reference RTN). absmax over each k-group's 32
    partitions via DVE 32x32 transpose + free-reduce; scale = 2^ceil(log2(
    absmax/6)) via integer exponent tricks; e2m1 RTN via r = (v+cs)-cs with
    cs = sign(v)|C, C = 1.5*2^M*max(ef(v),sc).
  - Phase order: W-quant -> AllGather (overlapped with X-quant) -> main
    matmul. W read once per source core as contiguous [128,KB,NPC] fp8.
  - Main matmul fp8 DoubleRow: yT[n,t] += wdq.T @ xdq over k, PSUM f32;
    evac with *1/(gsx*gsw) + bias on ACT; host reassembles.
"""
import os

import numpy as np

GROUP = 32
B, S, K, N = 4, 2048, 4096, 4096
TOK = B * S
NCORES = 8
TPC = TOK // NCORES      # tokens per core
NPC = N // NCORES        # n-columns quantized per core

_prog_cache = {}
LAST_EXEC_NS = None
LAST_RESULTS = None


def _build_program(n_splits):
    import concourse.bass as bass
    import concourse.mybir as mybir
    import concourse.tile as tile
    from concourse import bacc

    F32 = mybir.dt.float32
    F16 = mybir.dt.float16
    FP8 = mybir.dt.float8e4
    I32 = mybir.dt.int32
    I16 = mybir.dt.int16
    Alu = mybir.AluOpType
    Act = mybir.ActivationFunctionType

    KB = K // 128          # 32 k-blocks
    FDC = 512              # quant chunk free-dim
    chain16 = os.environ.get("KQ_CHAIN", "16") == "16"
    dbg = os.environ.get("KQ_DEBUG", "0") == "1"

    nc = bacc.Bacc(None, target_bir_lowering=False)

    xh_d = nc.dram_tensor("xhT", [K, TPC], F16, kind="ExternalInput")
    xl_d = nc.dram_tensor("xlT", [K, TPC], F16, kind="ExternalInput")
    wh_d = nc.dram_tensor("whT", [K, NPC], F16, kind="ExternalInput")
    wl_d = nc.dram_tensor("wlT", [K, NPC], F16, kind="ExternalInput")
    bdp_d = nc.dram_tensor("bdp", [n_splits, 128, 128], F16, kind="ExternalInput")
    bias_d = nc.dram_tensor("bias", [N], F32, kind="ExternalInput")
    scl_d = nc.dram_tensor("scl", [128, 1], F32, kind="ExternalInput")
    y_d = nc.dram_tensor("yT", [N, TPC], F32, kind="ExternalOutput")

    wsh_d = nc.dram_tensor("wdq_sh", [128, KB, NPC], FP8)
    wg_d = nc.dram_tensor("wdq_g", [NCORES, 128, KB, NPC], FP8,
                          addr_space="Shared")
    xdq_dbg = nc.dram_tensor("xdq_i", [128, KB, TPC], FP8,
                             kind="ExternalOutput") if dbg else None

    if chain16:
        DT, IT = F16, I16
        m_exp, m_sgn = 0x7C00, -0x8000
        ceil_add, c_add = 0x03FF, 0x2600
    else:
        DT, IT = F32, I32
        m_exp, m_sgn = 0x7F800000, -0x80000000
        ceil_add, c_add = 0x7FFFFF, 0x0B400000

    with tile.TileContext(nc) as tc:
        with (
            tc.tile_pool(name="singles", bufs=1) as singles,
            tc.tile_pool(name="stage", bufs=3) as stage,
            tc.tile_pool(name="qwork", bufs=2) as qwork,
            tc.tile_pool(name="wmain", bufs=2) as wmain,
            tc.tile_pool(name="ymain", bufs=3) as ymain,
            tc.tile_pool(name="vps", bufs=2, space="PSUM") as vps,
            tc.tile_pool(name="yps", bufs=2, space="PSUM") as yps,
        ):
            # ---------- constants ----------
            bdp_t = singles.tile([128, n_splits, 128], F16)
            for sp in range(n_splits):
                nc.sync.dma_start(bdp_t[:, sp, :], bdp_d[sp])
            bias_t = singles.tile([128, N // 128], F32)
            nc.sync.dma_start(bias_t[:], bias_d[:].rearrange("(a b) -> b a", b=128))
            scl_bc = singles.tile([128, 1], F32)
            nc.sync.dma_start(scl_bc[:], scl_d[:])
            sgnmask = singles.tile([128, 1], IT)
            nc.vector.memset(sgnmask[:], m_sgn)

            inv6 = float(np.float32(1.0) / np.float32(6.0))

            # ---------- quant pipeline for one [128, fd] chunk ----------
            def quant_chunk(h_t, l_t, fd, dq_out_ap):
                vp = vps.tile([128, FDC], F32, tag="vp")
                for sp in range(n_splits):
                    nc.tensor.matmul(
                        vp[:, :fd], bdp_t[:, sp, :], (h_t, l_t)[sp % 2][:, :fd],
                        start=(sp == 0), stop=(sp == n_splits - 1),
                    )
                vc = qwork.tile([128, FDC], DT, tag="vc")
                nc.scalar.activation(vc[:, :fd], vp[:, :fd], Act.Copy)
                # absmax over 32-partition groups via transpose32 + reduce
                t32 = qwork.tile([128, FDC], DT, tag="t32")
                nc.vector.transpose(t32[:, :fd], vc[:, :fd])
                rs = fd // 32
                red = qwork.tile([128, FDC // 32], DT, tag="red")
                nc.vector.tensor_reduce(
                    red[:, :rs],
                    t32[:, :fd].rearrange("p (j b) -> p j b", b=32),
                    mybir.AxisListType.X, Alu.max,
                    apply_absolute_value=True,
                )
                # sc = ceilpow2(red/6) exponent bits (small tensor)
                t2 = qwork.tile([128, FDC // 32], DT, tag="t2")
                nc.vector.tensor_scalar(t2[:, :rs], red[:, :rs], inv6, None,
                                        Alu.mult)
                nc.vector.tensor_scalar(t2[:, :rs].bitcast(IT),
                                        t2[:, :rs].bitcast(IT),
                                        ceil_add, None, Alu.add)
                nc.vector.tensor_scalar(t2[:, :rs].bitcast(IT),
                                        t2[:, :rs].bitcast(IT),
                                        m_exp, None, Alu.bitwise_and)
                scb = t2[:, :rs].unsqueeze(2).broadcast_to([128, rs, 32])
                # C = (max(ef(v), sc) bits) + 1.5*2^M step
                ea = qwork.tile([128, FDC], DT, tag="ea")
                nc.vector.tensor_scalar(
                    ea[:, :fd].bitcast(IT), t32[:, :fd].bitcast(IT),
                    m_exp, None, Alu.bitwise_and,
                )
                e = qwork.tile([128, FDC], DT, tag="e")
                nc.vector.tensor_tensor(
                    e[:, :fd].rearrange("p (j b) -> p j b", b=32).bitcast(IT),
                    ea[:, :fd].rearrange("p (j b) -> p j b", b=32).bitcast(IT),
                    scb.bitcast(IT), Alu.max,
                )
                C = qwork.tile([128, FDC], DT, tag="C")
                nc.vector.tensor_scalar(
                    C[:, :fd].bitcast(IT), e[:, :fd].bitcast(IT),
                    c_add, None, Alu.add,
                )
                cs = qwork.tile([128, FDC], DT, tag="cs")
                nc.vector.scalar_tensor_tensor(
                    cs[:, :fd].bitcast(IT), t32[:, :fd].bitcast(IT),
                    sgnmask[:, 0:1], C[:, :fd].bitcast(IT),
                    Alu.bitwise_and, Alu.bitwise_or,
                )
                t = qwork.tile([128, FDC], DT, tag="t")
                nc.vector.tensor_tensor(t[:, :fd], t32[:, :fd], cs[:, :fd],
                                        Alu.add)
                dq = qwork.tile([128, FDC], DT, tag="dq")
                nc.vector.tensor_tensor(dq[:, :fd], t[:, :fd], cs[:, :fd],
                                        Alu.subtract)
                dqt = qwork.tile([128, FDC], DT, tag="dqt")
                nc.vector.transpose(dqt[:, :fd], dq[:, :fd])
                nc.scalar.activation(dq_out_ap, dqt[:, :fd], Act.Copy)

            # ---------- phase 1: quantize this core's W n-slice ----------
            for kb in range(KB):
                h_t = stage.tile([128, FDC], F16, tag="wh")
                l_t = stage.tile([128, FDC], F16, tag="wl")
                nc.sync.dma_start(h_t[:], wh_d[kb * 128:(kb + 1) * 128, :])
                nc.sync.dma_start(l_t[:], wl_d[kb * 128:(kb + 1) * 128, :])
                dq8 = ymain.tile([128, FDC], FP8, tag="wdq8")
                quant_chunk(h_t, l_t, NPC, dq8[:, :NPC])
                nc.sync.dma_start(wsh_d[:, kb, :], dq8[:, :NPC])

            # ---------- all-gather W shards (overlaps X-quant below) ----------
            nc.gpsimd.collective_compute(
                "AllGather", mybir.AluOpType.bypass,
                replica_groups=[list(range(NCORES))],
                ins=[wsh_d[:]],
                outs=[wg_d[:]],
            )

            # ---------- phase 2: X-quant token-chunks ----------
            NSPLIT = int(os.environ.get("KQ_TSPLIT", "2"))
            TW = TPC // NSPLIT
            xdq_ts = [singles.tile([128, KB, TW], FP8, name=f"xdq{i}",
                                   tag=f"xdq{i}")
                      for i in range(NSPLIT)]
            for tci in range(NSPLIT):
                xdq_c = xdq_ts[tci]
                toff = tci * TW
                for kb in range(KB):
                    for ch in range(TW // FDC):
                        csl = slice(toff + ch * FDC, toff + (ch + 1) * FDC)
                        h_t = stage.tile([128, FDC], F16, tag="xh")
                        l_t = stage.tile([128, FDC], F16, tag="xl")
                        nc.sync.dma_start(h_t[:], xh_d[kb * 128:(kb + 1) * 128, csl])
                        nc.sync.dma_start(l_t[:], xl_d[kb * 128:(kb + 1) * 128, csl])
                        quant_chunk(h_t, l_t, FDC,
                                    xdq_c[:, kb, ch * FDC:(ch + 1) * FDC])
                if dbg:
                    nc.sync.dma_start(xdq_dbg[:, :, toff:toff + TW], xdq_c[:])

            # ---------- phase 3: main matmul, W read once per source core ----
            for c in range(NCORES):
                wblk = wmain.tile([128, KB, NPC], FP8, tag="wblk")
                nc.sync.dma_start(wblk[:], wg_d[c])
                for tci in range(NSPLIT):
                    xdq_c = xdq_ts[tci]
                    for ns in range(NPC // 128):
                        nt = c * (NPC // 128) + ns
                        yp = yps.tile([128, TW], F32, tag="yp")
                        for kb2 in range(KB // 2):
                            nc.tensor.matmul(
                                yp[:],
                                wblk[:, 2 * kb2:2 * kb2 + 2,
                                     ns * 128:(ns + 1) * 128],
                                xdq_c[:, 2 * kb2:2 * kb2 + 2, :],
                                start=(kb2 == 0), stop=(kb2 == KB // 2 - 1),
                                perf_mode=mybir.MatmulPerfMode.DoubleRow,
                            )
                        ysb = ymain.tile([128, TW], F32, tag="ysb")
                        nc.scalar.activation(
                            ysb[:], yp[:], Act.Identity,
                            bias=bias_t[:, nt:nt + 1], scale=scl_bc[:, 0:1],
                        )
                        nc.sync.dma_start(
                            y_d[nt * 128:(nt + 1) * 128,
                                tci * TW:(tci + 1) * TW], ysb[:])

    nc.compile()
    return nc


def _get_program(n_splits):
    key = (n_splits, os.environ.get("KQ_CHAIN", "16"),
           os.environ.get("KQ_TSPLIT", "2"), os.environ.get("KQ_DEBUG", "0"))
    if key not in _prog_cache:
        _prog_cache[key] = _build_program(n_splits)
    return _prog_cache[key]


def _prepare(x, weight, bias, hadamard_matrix, weight_global_scale, act_global_scale):
    x = np.asarray(x, dtype=np.float32)
    weight = np.asarray(weight, dtype=np.float32)
    bias = np.asarray(bias, dtype=np.float32)
    H = np.asarray(hadamard_matrix, dtype=np.float32)
    gsw = np.float32(np.asarray(weight_global_scale).reshape(-1)[0])
    gsx = np.float32(np.asarray(act_global_scale).reshape(-1)[0])

    s = np.float32(np.abs(H).max())
    Pm = (H / s).astype(np.float32)
    Ph = Pm.astype(np.float16)
    Pl = (Pm - Ph.astype(np.float32)).astype(np.float16)
    generic = bool(np.any(Pl != 0))
    n_splits = 4 if generic else 2

    eye4 = np.eye(4, dtype=np.float32)
    if generic:
        bdp = np.stack([
            np.kron(eye4, Ph.astype(np.float32)).astype(np.float16),
            np.kron(eye4, Ph.astype(np.float32)).astype(np.float16),
            np.kron(eye4, Pl.astype(np.float32)).astype(np.float16),
            np.kron(eye4, Pl.astype(np.float32)).astype(np.float16),
        ])
    else:
        bdp = np.stack([np.kron(eye4, Ph.astype(np.float32)).astype(np.float16)] * 2)

    cx = np.float32(gsx * s)
    cw = np.float32(gsw * s)
    inv_gs = np.full((128, 1), np.float32(1.0) / np.float32(gsx * gsw),
                     dtype=np.float32)

    xs = (x.reshape(TOK, K) * cx).astype(np.float32)
    xh = xs.astype(np.float16)
    xl = (xs - xh.astype(np.float32)).astype(np.float16)

    ws = (weight * cw).astype(np.float32)
    wh = ws.astype(np.float16)
    wl = (ws - wh.astype(np.float32)).astype(np.float16)
    whT = np.ascontiguousarray(wh.T)
    wlT = np.ascontiguousarray(wl.T)

    nc = _get_program(n_splits)

    in_maps = []
    for c in range(NCORES):
        tsl = slice(c * TPC, (c + 1) * TPC)
        nslc = slice(c * NPC, (c + 1) * NPC)
        in_maps.append({
            "xhT": np.ascontiguousarray(xh[tsl].T),
            "xlT": np.ascontiguousarray(xl[tsl].T),
            "whT": np.ascontiguousarray(whT[:, nslc]),
            "wlT": np.ascontiguousarray(wlT[:, nslc]),
            "bdp": bdp,
            "bias": bias,
            "scl": inv_gs,
        })
    return nc, in_maps


def _assemble(results):
    y = np.empty((TOK, N), dtype=np.float32)
    for c in range(NCORES):
        y[c * TPC:(c + 1) * TPC] = results[c]["yT"].T
    return y.reshape(B, S, N)


def kernel(x, weight, bias, hadamard_matrix, weight_global_scale, act_global_scale):
    from concourse.bass_utils import run_bass_kernel_spmd

    nc, in_maps = _prepare(x, weight, bias, hadamard_matrix,
                           weight_global_scale, act_global_scale)
    trace = os.environ.get("KQ_TRACE", "0") == "1"
    res = run_bass_kernel_spmd(nc, in_maps, list(range(NCORES)), trace=trace,
                               tmpdir=os.environ.get("KQ_TRACE_DIR") or None)
    if trace:
        global LAST_EXEC_NS
        LAST_EXEC_NS = res.exec_time_ns
        print("exec_time_ns:", res.exec_time_ns, "mean:", res.mean_exec_time_ns)
    global LAST_RESULTS
    LAST_RESULTS = res.results
    return _assemble(res.results)
